# revision 1
# baseline (speedup 1.0000x reference)
"""Trainium2 Bass kernel for nn_AffinityPredictor (sparse voxel GNN).

Strategy: shard N=65536 points across 8 cores (8192/core). Exploit mask
sparsity (~1.79 real neighbors of 27) with per-offset edge lists built on
host. Per conv layer, per core:
  1. transposing fp16 dma_gather pulls neighbor features channel-major
  2. weight-stationary fp16 matmuls produce per-edge outputs Z (f32 psum)
  3. Z roundtrips DRAM; a second dma_gather permutes Z rows into dst order
  4. per-dst-tile segmented-sum matmuls (one-hot Seg built on device)
  5. BN stats per shard -> AllReduce [2,256] -> fused BN/relu/residual pass
Halo handling: redundant compute with per-layer shrinking windows (no
feature exchange between cores).
"""
import numpy as np

N = 65536
NC = 8
M = N // NC
K = 27
CH = 256
CO = 128
EPS = 1e-5
NLAYER = 9          # conv layers: L0 input conv, L1..L8 residual convs
GC1B = 384          # idxs/call, transpose gather of 512B rows (ring cap)
GC1S = 896          # idxs/call, transpose gather of 256B rows
GC2 = 896           # idxs/call, non-transpose gather of 512B rows
ZB = 16             # Z-write batch (echunks per DMA)
PB = 8              # park/table batch (tiles per DMA)
MAXG1 = 32640       # max gather-1 rows per k-split group (int16 limit)

_CACHE = {}


def _wrap16(idx):
    """dma_gather index layout: [128, n/16] int16, idx i at [i%16, i//16],
    replicated to all 8 gpsimd cores (partitions 16c+j)."""
    n = len(idx)
    assert n % 16 == 0
    w = np.ascontiguousarray(idx.astype(np.int16).reshape(-1, 16).T)
    return np.tile(w, (8, 1))


def _rup(x, m):
    return (x + m - 1) // m * m


def preprocess(feats, w_in, g_in, b_in, w_res, g_res, b_res, w_out,
               nbr_idx, nbr_mask):
    idx = np.asarray(nbr_idx)
    mask = np.asarray(nbr_mask).astype(bool)
    feats = np.asarray(feats, np.float32)

    # halo quantum from actual data
    delta = np.abs(idx.astype(np.int64) - np.arange(N)[:, None])[mask]
    maxd = int(delta.max()) if delta.size else 0
    Hq = max(_rup(maxd + 1, 128), 128)
    assert Hq <= 2048, f"non-local neighbor structure (Hq={Hq}) unsupported"

    pad = [(8 - l) * Hq for l in range(NLAYER)]          # dst-window halo
    tpad = [(9 - l) * Hq for l in range(NLAYER)]         # table halo
    win = [M + 2 * pad[l] for l in range(NLAYER)]        # dst rows per layer
    Rtab = [M + 2 * tpad[l] for l in range(NLAYER)] + [M]

    per_core = [dict() for _ in range(NC)]
    meta = {"Hq": Hq, "pad": pad, "tpad": tpad, "win": win, "Rtab": Rtab}

    # ---- per-layer edge lists ----
    # per (layer, core): for k in 27: srcs (table-local), dsts (window-local)
    edges = [[None] * NC for _ in range(NLAYER)]
    for l in range(NLAYER):
        for s in range(NC):
            wlo = s * M - pad[l]
            tlo = s * M - tpad[l]
            rows = np.arange(wlo, wlo + win[l])
            valid = (rows >= 0) & (rows < N)
            rv = rows[valid]
            mk = mask[rv]                                # [nv, K]
            ik = idx[rv]
            ks, kd = [], []
            for k in range(K):
                sel = mk[:, k]
                ks.append((ik[sel, k] - tlo).astype(np.int32))
                kd.append((rv[sel] - wlo).astype(np.int32))
            edges[l][s] = (ks, kd)

    # ---- gather-1 lists (k-grouped, per-k padded, split into groups) ----
    # group splitting by k so each group's padded row count <= MAXG1
    for l in range(NLAYER):
        ekmax = [max(_rup(max(len(edges[l][s][0][k]) for s in range(NC)), 128), 128)
                 for k in range(K)]
        groups = []   # list of list of k
        cur, cnt = [], 0
        for k in range(K):
            if cur and cnt + ekmax[k] > MAXG1:
                groups.append(cur)
                cur, cnt = [], 0
            cur.append(k)
            cnt += ekmax[k]
        groups.append(cur)
        meta.setdefault("groups", {})[l] = groups
        meta.setdefault("ekmax", {})[l] = ekmax

        for s in range(NC):
            ks, kd = edges[l][s]
            g1 = []          # per group: int32 array of table rows
            zdst = []        # per group: window-local dst per Z row (-1 pad)
            for g in groups:
                parts, dparts = [], []
                for k in g:
                    e = ks[k]
                    p = np.zeros(ekmax[k], np.int32)
                    p[:len(e)] = e
                    parts.append(p)
                    d = np.full(ekmax[k], -1, np.int32)
                    d[:len(e)] = kd[k]
                    dparts.append(d)
                g1.append(np.concatenate(parts))
                zdst.append(np.concatenate(dparts))
            per_core[s].setdefault("g1", {})[l] = g1
            per_core[s].setdefault("zdst", {})[l] = zdst

    # ---- gather-2 lists + dstof (per tile, per group) ----
    # uniform chunk counts across cores: per (l, tile, group)
    for l in range(NLAYER):
        ntile = win[l] // 128
        ngrp = len(meta["groups"][l])
        # count per core/tile/group
        cnts = np.zeros((NC, ntile, ngrp), np.int64)
        for s in range(NC):
            for g in range(ngrp):
                zd = per_core[s]["zdst"][l][g]
                t = zd[zd >= 0] // 128
                np.add.at(cnts[s, :, g], t, 1)
        chunks = np.maximum(_rup(cnts.max(0), 128) // 128, 1)  # [ntile, ngrp]
        meta.setdefault("chunks", {})[l] = chunks
        for s in range(NC):
            g2 = []
            dstof = []
            for g in range(ngrp):
                zd = per_core[s]["zdst"][l][g]
                order = np.argsort(zd[zd >= 0] // 128, kind="stable")
                zr = np.nonzero(zd >= 0)[0][order]      # z rows sorted by tile
                dd = zd[zr]
                lst = np.zeros(int(chunks[:, g].sum()) * 128, np.int32)
                dof = np.full(lst.shape, -1, np.float16)
                off = 0
                ptr = 0
                for t in range(ntile):
                    n_t = int(cnts[s, t, g])
                    cap = int(chunks[t, g]) * 128
                    lst[off:off + n_t] = zr[ptr:ptr + n_t]
                    dof[off:off + n_t] = (dd[ptr:ptr + n_t] - t * 128).astype(np.float16)
                    ptr += n_t
                    off += cap
                g2.append(lst)
                dstof.append(dof)
            per_core[s].setdefault("g2", {})[l] = g2
            per_core[s].setdefault("dstof", {})[l] = dstof

    # ---- input staging arrays ----
    f16 = np.float16
    ins = []
    # weights
    w0 = np.zeros((K, 128, CH), f16)
    w0[:, :3, :] = np.asarray(w_in, np.float32).astype(f16)
    wr = np.asarray(w_res, np.float32).reshape(8, K, 2, 128, CH) \
        .transpose(0, 1, 3, 2, 4).astype(f16)          # [8,K,128,2,256]
    wo = np.asarray(w_out, np.float32).reshape(2, 128, CO) \
        .transpose(1, 0, 2).astype(f16)                # [128,2,128]
    bng = np.zeros((NLAYER, CH), np.float32)
    bnb = np.zeros((NLAYER, CH), np.float32)
    bng[0], bnb[0] = np.asarray(g_in), np.asarray(b_in)
    gr = np.asarray(g_res, np.float32).reshape(8, CH)
    br = np.asarray(b_res, np.float32).reshape(8, CH)
    bng[1:], bnb[1:] = gr, br

    for s in range(NC):
        d = {"w0": w0, "wr": wr, "wo": wo, "bng": bng, "bnb": bnb,
             "ones16": np.ones((128, 1), f16),
             "ones32": np.ones((1, 128), np.float32),
             "iota": np.tile(np.arange(128, dtype=f16), (128, 1)),
             "ident128": np.eye(128, dtype=f16)}
        # feats table, padded to 128 wide
        tlo = s * M - tpad[0]
        t0 = np.zeros((Rtab[0], 128), f16)
        lo, hi = max(0, tlo), min(N, tlo + Rtab[0])
        t0[lo - tlo:hi - tlo, :3] = feats[lo:hi].astype(f16)
        d["tab0"] = t0
        for l in range(NLAYER):
            for g in range(len(meta["groups"][l])):
                d[f"g1_{l}_{g}"] = _wrap16(per_core[s]["g1"][l][g])
                d[f"g2_{l}_{g}"] = _wrap16(per_core[s]["g2"][l][g])
                # dstof: [128, nchunk] fp16, z-lane major
                dof = per_core[s]["dstof"][l][g].reshape(-1, 128).T
                d[f"dof_{l}_{g}"] = np.ascontiguousarray(dof)
        ins.append(d)
    return ins, meta


def build_program(meta):
    import os
    klayers = int(os.environ.get("KLAYERS", str(NLAYER)))
    import concourse.tile as tile
    from concourse import bass, bacc, mybir
    from contextlib import ExitStack
    f16, f32, i16 = mybir.dt.float16, mybir.dt.float32, mybir.dt.int16

    pad, tpad, win, Rtab = meta["pad"], meta["tpad"], meta["win"], meta["Rtab"]
    groups, ekmax, chunks = meta["groups"], meta["ekmax"], meta["chunks"]
    Hq = meta["Hq"]

    nc = bacc.Bacc("TRN2", target_bir_lowering=False, debug=False,
                   num_devices=NC, num_swdge_queues=2)

    def din(name, shape, dt):
        return nc.dram_tensor(name, list(shape), dt, kind="ExternalInput").ap()

    w0 = din("w0", (K, 128, CH), f16)
    wr = din("wr", (8, K, 128, 2, CH), f16)
    wo = din("wo", (128, 2, CO), f16)
    bng = din("bng", (NLAYER, CH), f32)
    bnb = din("bnb", (NLAYER, CH), f32)
    ones16 = din("ones16", (128, 1), f16)
    ones32 = din("ones32", (1, 128), f32)
    iota_in = din("iota", (128, 128), f16)
    ident_in = din("ident128", (128, 128), f16)
    tab0 = din("tab0", (Rtab[0], 128), f16)
    idf = din("idf", (128, M // 16), i16)
    g1_in, g2_in, dof_in = {}, {}, {}
    for l in range(NLAYER):
        for g in range(len(groups[l])):
            e1 = len_g1(meta, l, g)
            e2 = int(chunks[l][:, g].sum()) * 128
            g1_in[(l, g)] = din(f"g1_{l}_{g}", (128, e1 // 16), i16)
            g2_in[(l, g)] = din(f"g2_{l}_{g}", (128, e2 // 16), i16)
            dof_in[(l, g)] = din(f"dof_{l}_{g}", (128, e2 // 128), f16)

    out_t = nc.dram_tensor("out", [M, CO], f32, kind="ExternalOutput").ap()
    dbg_n = int(os.environ.get("DBGTAPS", "0"))
    dbg_park = [nc.dram_tensor(f"dbgp{i}", [M, CH], f16, kind="ExternalOutput").ap()
                for i in range(dbg_n)]
    dbg_z = nc.dram_tensor("dbgz", [M, CH], f16, kind="ExternalOutput").ap() \
        if dbg_n else None
    dbg_g = nc.dram_tensor("dbgg", [128, GC1S], f16, kind="ExternalOutput").ap() \
        if dbg_n else None
    dbg_tab = [nc.dram_tensor(f"dbgt{i}", [M, CH], f16, kind="ExternalOutput").ap()
               for i in range(dbg_n)]

    with tile.TileContext(nc) as tc, ExitStack() as ctx:
        sb = ctx.enter_context(tc.tile_pool(name="sb", bufs=2))
        sbc = ctx.enter_context(tc.tile_pool(name="sbc", bufs=1))   # constants
        gpool = ctx.enter_context(tc.tile_pool(name="gpool", bufs=3))
        wpool = ctx.enter_context(tc.tile_pool(name="wpool", bufs=3))
        zpool = ctx.enter_context(tc.tile_pool(name="zpool", bufs=3))
        spool = ctx.enter_context(tc.tile_pool(name="spool", bufs=4))
        ppool = ctx.enter_context(tc.tile_pool(name="ppool", bufs=3, space="PSUM"))
        pso = ctx.enter_context(tc.tile_pool(name="pso", bufs=2, space="PSUM"))
        pstat = ctx.enter_context(tc.tile_pool(name="pstat", bufs=1, space="PSUM"))
        dram = ctx.enter_context(tc.tile_pool(name="dram", bufs=1, space="DRAM"))

        # persistent constants
        onesK = sbc.tile([128, 1], f16)
        nc.sync.dma_start(out=onesK[:], in_=ones16[:])
        ones1 = sbc.tile([1, 128], f32)
        nc.sync.dma_start(out=ones1[:], in_=ones32[:])
        iota = sbc.tile([128, 128], f16)
        nc.sync.dma_start(out=iota[:], in_=iota_in[:])
        ident = sbc.tile([128, 128], f16)
        nc.sync.dma_start(out=ident[:], in_=ident_in[:])

        # DRAM scratch
        tabs = [tab0] + [dram.tile([Rtab[l], CH], f16, name=f"tabi{l}")
                         for l in range(1, NLAYER + 1)]
        maxg1 = max(sum(len_g1(meta, l, g) for g in range(len(groups[l])))
                    for l in range(NLAYER))
        zdram = [dram.tile([maxg1, CH], f16, name=f"zd{i}") for i in range(2)]
        maxwin = max(win)
        parks = [dram.tile([maxwin, CH], f16, name=f"park{i}") for i in range(2)]

        def gather1(table, idx_t, e1, out_tiles, elem, dbg_first=False):
            """transposing gather of e1 rows from table; returns list of
            (tile, ncols) gbuf tiles"""
            res = []
            gc = GC2
            for c0 in range(0, e1, gc):
                n = min(gc, e1 - c0)
                gb = gpool.tile([128, n // 128, elem], f16, tag="gbuf")
                nc.gpsimd.dma_gather(
                    out_ap=gb[:], in_ap=table[:],
                    idxs_ap=idx_t[:, c0 // 16:(c0 + n) // 16],
                    num_idxs=n, num_idxs_reg=n, elem_size=elem)
                res.append((gb, n))
                if dbg_n and dbg_first and c0 == 0:
                    nc.sync.dma_start(out=dbg_g[:, :n], in_=gb[:, 0, :])
            return res

        def load_idx(src, e):
            t = sb.tile([128, e // 16], i16, tag="idx")
            nc.sync.dma_start(out=t[:], in_=src[:])
            return t

        for l in range(klayers):
            table = tabs[l]
            zcur = zdram[l % 2]
            park = parks[l % 2]
            elem = 128 if l == 0 else CH
            nblk = 1 if l == 0 else 2
            ntile = win[l] // 128
            own0 = pad[l] // 128

            # ---- conv GEMMs, k-grouped ----
            zrow0 = 0
            for g, gks in enumerate(groups[l]):
                e1 = len_g1(meta, l, g)
                idx_t = load_idx(g1_in[(l, g)], e1)
                gbufs = gather1(table, idx_t, e1, None, elem,
                                dbg_first=(dbg_n and l == 0 and g == 0))
                # per-k weight tiles and echunk matmuls
                ech = 0          # echunk index within group
                zst = None
                zst_n = 0
                zst_base = 0
                for k in gks:
                    wk = wpool.tile([128, nblk, CH], f16, tag="wk")
                    if l == 0:
                        nc.sync.dma_start(out=wk[:, 0, :], in_=w0[k])
                    else:
                        nc.sync.dma_start(out=wk[:], in_=wr[l - 1, k])
                    for _ in range(ekmax[l][k] // 128):
                        call, off = divmod(ech * 128, GC2)
                        gb = gbufs[call][0]
                        gcm = spool.tile([128, nblk, 128], f16, tag="gcm")
                        for b in range(nblk):
                            pt = ppool.tile([128, 128], f16, space="PSUM",
                                            tag="pt", bufs=2)
                            nc.tensor.transpose(
                                out=pt[:],
                                in_=gb[:, off // 128, b * 128:(b + 1) * 128],
                                identity=ident[:])
                            nc.any.tensor_copy(out=gcm[:, b, :], in_=pt[:])
                        pz = ppool.tile([128, CH], f32, space="PSUM", tag="pz")
                        for b in range(nblk):
                            nc.tensor.matmul(
                                out=pz[:], lhsT=gcm[:, b, :],
                                rhs=wk[:, b, :], start=(b == 0),
                                stop=(b == nblk - 1))
                        if zst is None:
                            zst = zpool.tile([128, ZB, CH], f16, tag="zst")
                            zst_base = ech
                            zst_n = 0
                        nc.any.tensor_copy(out=zst[:, zst_n, :], in_=pz[:])
                        zst_n += 1
                        ech += 1
                        if zst_n == ZB:
                            dst = zcur[zrow0 + zst_base * 128:
                                       zrow0 + (zst_base + ZB) * 128, :]
                            nc.sync.dma_start(
                                out=dst.rearrange("(b p) f -> p b f", p=128),
                                in_=zst[:])
                            zst = None
                if zst is not None:
                    dst = zcur[zrow0 + zst_base * 128:
                               zrow0 + (zst_base + zst_n) * 128, :]
                    nc.sync.dma_start(
                        out=dst.rearrange("(b p) f -> p b f", p=128),
                        in_=zst[:, :zst_n, :])
                zrow0 += e1

            # ---- gather-2 + Seg aggregation + stats ----
            pst = pstat.tile([1, 2 * CH], f32, space="PSUM", tag="ps")
            ngrp = len(groups[l])
            # z-permute gathers: emitted lazily, interleaved with consumption
            zs_bufs = {g: [] for g in range(ngrp)}
            dof_bufs = {}
            idx2_t = {}
            e2s = {}
            zoffs = {}
            zoff = 0
            for g in range(ngrp):
                e2s[g] = int(chunks[l][:, g].sum()) * 128
                idx2_t[g] = load_idx(g2_in[(l, g)], e2s[g])
                dof = sb.tile([128, e2s[g] // 128], f16, tag=f"dof{g}")
                nc.sync.dma_start(out=dof[:], in_=dof_in[(l, g)][:])
                dof_bufs[g] = dof
                zoffs[g] = zoff
                zoff += len_g1(meta, l, g)

            def emit_g2(g, call):
                c0 = call * GC2
                n = min(GC2, e2s[g] - c0)
                zt = zpool.tile([128, n // 128, CH], f16, tag=f"zs{g}",
                                name=f"zt{g}")
                nc.gpsimd.dma_gather(
                    out_ap=zt[:],
                    in_ap=zdram_sl(zcur, zoffs[g], len_g1(meta, l, g)),
                    idxs_ap=idx2_t[g][:, c0 // 16:(c0 + n) // 16],
                    num_idxs=n, num_idxs_reg=n, elem_size=CH,
                    queue_num=1)
                zs_bufs[g].append(zt)

            first_stat = True
            gchunk_pos = [0] * ngrp
            # residual source rows (block-end layers 2,4,6,8)
            is_res = (l >= 1) and (l % 2 == 0)
            res_tab = tabs[l - 1] if is_res else None
            res_off = 2 * Hq if is_res else 0

            nparked = 0
            park_st = None
            park_t0 = 0
            for t in range(ntile):
                po = pso.tile([128, CH], f32, space="PSUM", tag="po")
                first = True
                for g in range(ngrp):
                    for c in range(int(chunks[l][t, g])):
                        ci = gchunk_pos[g]
                        gchunk_pos[g] += 1
                        call, slot = divmod(ci * 128, GC2)
                        while len(zs_bufs[g]) <= call:
                            emit_g2(g, len(zs_bufs[g]))
                        seg = spool.tile([128, 128], f16, tag="seg")
                        nc.vector.tensor_tensor(
                            out=seg[:],
                            in0=dof_bufs[g][:, ci:ci + 1].to_broadcast([128, 128]),
                            in1=iota[:],
                            op=mybir.AluOpType.is_equal)
                        last = (g == ngrp - 1) and (c == int(chunks[l][t, g]) - 1)
                        nc.tensor.matmul(
                            out=po[:], lhsT=seg[:],
                            rhs=zs_bufs[g][call][:, slot // 128, :],
                            start=first, stop=last)
                        first = False
                if park_st is None:
                    park_st = zpool.tile([128, PB, CH], f16, tag="pk")
                    park_t0 = t
                pslot = t - park_t0
                nc.any.tensor_copy(out=park_st[:, pslot, :], in_=po[:])
                if own0 <= t < own0 + M // 128:
                    last_stat = (t == own0 + M // 128 - 1)
                    nc.tensor.matmul(out=pst[:, :CH], lhsT=onesK[:],
                                     rhs=park_st[:, pslot, :],
                                     start=first_stat, stop=last_stat,
                                     skip_group_check=True)
                    sq = spool.tile([128, CH], f16, tag="sq")
                    nc.vector.tensor_tensor(out=sq[:], in0=park_st[:, pslot, :],
                                            in1=park_st[:, pslot, :],
                                            op=mybir.AluOpType.mult)
                    nc.tensor.matmul(out=pst[:, CH:], lhsT=onesK[:], rhs=sq[:],
                                     start=first_stat, stop=last_stat,
                                     skip_group_check=True)
                    first_stat = False
                if pslot == PB - 1 or t == ntile - 1:
                    dst = park[park_t0 * 128:(t + 1) * 128, :]
                    nc.sync.dma_start(
                        out=dst.rearrange("(b p) f -> p b f", p=128),
                        in_=park_st[:, :pslot + 1, :])
                    park_st = None

            # ---- stats allreduce + BN coefficients ----
            ccin = dram.tile([2, CH], f32, name=f"ccin{l}")
            ccout = dram.tile([2, CH], f32, addr_space="Shared", name=f"ccout{l}")
            s1t = sb.tile([1, CH], f32, tag="s1t")
            nc.vector.tensor_copy(out=s1t[:], in_=pst[:, :CH])
            s2t = sb.tile([1, CH], f32, tag="s2t")
            nc.vector.tensor_copy(out=s2t[:], in_=pst[:, CH:])
            nc.sync.dma_start(out=ccin[0:1, :], in_=s1t[:])
            nc.sync.dma_start(out=ccin[1:2, :], in_=s2t[:])
            nc.gpsimd.collective_compute(
                "AllReduce", mybir.AluOpType.add,
                ins=[ccin.opt()], outs=[ccout.opt()],
                replica_groups=[list(range(NC))])
            stg1 = sb.tile([1, CH], f32, tag="stg1")
            nc.sync.dma_start(out=stg1[:], in_=ccout[0:1, :])
            stg2 = sb.tile([1, CH], f32, tag="stg2")
            nc.sync.dma_start(out=stg2[:], in_=ccout[1:2, :])

            mu = sb.tile([1, CH], f32, tag="mu")
            nc.vector.tensor_scalar_mul(out=mu[:], in0=stg1[:],
                                        scalar1=1.0 / N)
            var = sb.tile([1, CH], f32, tag="var")
            nc.vector.tensor_scalar_mul(out=var[:], in0=stg2[:],
                                        scalar1=1.0 / N)
            mu2 = sb.tile([1, CH], f32, tag="mu2")
            nc.vector.tensor_tensor(out=mu2[:], in0=mu[:], in1=mu[:],
                                    op=mybir.AluOpType.mult)
            nc.vector.tensor_tensor(out=var[:], in0=var[:], in1=mu2[:],
                                    op=mybir.AluOpType.subtract)
            nc.vector.tensor_scalar_add(out=var[:], in0=var[:], scalar1=EPS)
            sd = sb.tile([1, CH], f32, tag="sd")
            nc.scalar.activation(out=sd[:], in_=var[:],
                                 func=mybir.ActivationFunctionType.Sqrt)
            # rsqrt with one Newton step (ACT sqrt is approximate):
            # r0 = 1/sd; r1 = r0*(1.5 - 0.5*var*r0^2)
            rsd = sb.tile([1, CH], f32, tag="rsd")
            nc.vector.reciprocal(out=rsd[:], in_=sd[:])
            q = sb.tile([1, CH], f32, tag="q")
            nc.vector.tensor_tensor(out=q[:], in0=rsd[:], in1=rsd[:],
                                    op=mybir.AluOpType.mult)
            nc.vector.tensor_tensor(out=q[:], in0=q[:], in1=var[:],
                                    op=mybir.AluOpType.mult)
            nc.vector.tensor_scalar(out=q[:], in0=q[:], scalar1=-0.5,
                                    scalar2=1.5, op0=mybir.AluOpType.mult,
                                    op1=mybir.AluOpType.add)
            nc.vector.tensor_tensor(out=rsd[:], in0=rsd[:], in1=q[:],
                                    op=mybir.AluOpType.mult)
            gv = sb.tile([1, CH], f32, tag="gv")
            nc.sync.dma_start(out=gv[:], in_=bng[l:l + 1, :])
            bv = sb.tile([1, CH], f32, tag="bv")
            nc.sync.dma_start(out=bv[:], in_=bnb[l:l + 1, :])
            sc = sb.tile([1, CH], f32, tag="sc")
            nc.vector.tensor_tensor(out=sc[:], in0=gv[:], in1=rsd[:],
                                    op=mybir.AluOpType.mult)
            tc_ = sb.tile([1, CH], f32, tag="tc_")
            nc.vector.tensor_tensor(out=tc_[:], in0=mu[:], in1=sc[:],
                                    op=mybir.AluOpType.mult)
            nc.vector.tensor_tensor(out=tc_[:], in0=bv[:], in1=tc_[:],
                                    op=mybir.AluOpType.subtract)
            # broadcast to 128 partitions
            pbc = pso.tile([128, CH], f32, space="PSUM", tag="po")
            nc.tensor.matmul(out=pbc[:], lhsT=ones1[:], rhs=sc[:],
                             start=True, stop=True)
            sbc_t = sb.tile([128, CH], f32, tag="sbct")
            nc.vector.tensor_copy(out=sbc_t[:], in_=pbc[:])
            pbc2 = pso.tile([128, CH], f32, space="PSUM", tag="po")
            nc.tensor.matmul(out=pbc2[:], lhsT=ones1[:], rhs=tc_[:],
                             start=True, stop=True)
            tbc_t = sb.tile([128, CH], f32, tag="tbct")
            nc.vector.tensor_copy(out=tbc_t[:], in_=pbc2[:])

            # ---- BN / relu / residual pass -> next table ----
            ttab = tabs[l + 1]
            for t0 in range(0, ntile, PB):
                nb = min(PB, ntile - t0)
                x = zpool.tile([128, PB, CH], f16, tag="bnx")
                src = park[t0 * 128:(t0 + nb) * 128, :]
                nc.sync.dma_start(
                    out=x[:, :nb, :],
                    in_=src.rearrange("(b p) f -> p b f", p=128))
                if is_res:
                    r = zpool.tile([128, PB, CH], f16, tag="bnr")
                    rsrc = res_tab[res_off + t0 * 128:res_off + (t0 + nb) * 128, :]
                    nc.sync.dma_start(
                        out=r[:, :nb, :],
                        in_=rsrc.rearrange("(b p) f -> p b f", p=128))
                y = zpool.tile([128, PB, CH], f16, tag="bny")
                for b in range(nb):
                    y32 = spool.tile([128, CH], f32, tag="y32")
                    nc.vector.tensor_tensor(out=y32[:], in0=x[:, b, :],
                                            in1=sbc_t[:],
                                            op=mybir.AluOpType.mult)
                    nc.vector.tensor_tensor(out=y32[:], in0=y32[:],
                                            in1=tbc_t[:],
                                            op=mybir.AluOpType.add)
                    if is_res:
                        nc.vector.tensor_tensor(out=y32[:], in0=y32[:],
                                                in1=r[:, b, :],
                                                op=mybir.AluOpType.add)
                    nc.scalar.activation(
                        out=y[:, b, :], in_=y32[:],
                        func=mybir.ActivationFunctionType.Relu)
                dst = ttab[t0 * 128:(t0 + nb) * 128, :]
                nc.sync.dma_start(
                    out=dst.rearrange("(b p) f -> p b f", p=128),
                    in_=y[:, :nb, :])

        if dbg_n and klayers >= 1:
            nc.sync.dma_start(out=dbg_z[:], in_=zdram[0][0:M, :])
        for i in range(dbg_n):
            if i < klayers:
                o = pad[i] * CH * 2
                nc.sync.dma_start(out=dbg_park[i][:],
                                  in_=parks[i % 2][pad[i]:pad[i] + M, :])
                nc.sync.dma_start(out=dbg_tab[i][:],
                                  in_=tabs[i + 1][pad[i]:pad[i] + M, :])
        # ---- output conv: out = h @ w_out ----
        wof = sbc.tile([128, 2, CO], f16)
        nc.sync.dma_start(out=wof[:], in_=wo[:])
        idf_t = load_idx(idf, M)
        for c0 in (range(0, M, GC1B) if klayers == NLAYER else []):
            nn = min(GC1B, M - c0)
            gb = gpool.tile([128, 2, nn], f16, tag="gbuf")
            nc.gpsimd.dma_gather(
                out_ap=gb[:], in_ap=tabs[NLAYER][:],
                idxs_ap=idf_t[:, c0 // 16:(c0 + nn) // 16],
                num_idxs=nn, num_idxs_reg=nn, elem_size=CH,
                transpose=True)
            for j in range(nn // 128):
                pz = ppool.tile([128, CO], f32, space="PSUM", tag="pz")
                for b in range(2):
                    nc.tensor.matmul(out=pz[:], lhsT=gb[:, b, j * 128:(j + 1) * 128],
                                     rhs=wof[:, b, :], start=(b == 0),
                                     stop=(b == 1))
                ot = spool.tile([128, CO], f32, tag="ot")
                nc.any.tensor_copy(out=ot[:], in_=pz[:])
                nc.sync.dma_start(
                    out=out_t[c0 + j * 128:c0 + (j + 1) * 128, :], in_=ot[:])

    nc.finalize()
    return nc


def len_g1(meta, l, g):
    return sum(meta["ekmax"][l][k] for k in meta["groups"][l][g])


def zdram_sl(z, off, n):
    return z[off:off + n, :]


def kernel(**inputs):
    from concourse.bass_utils import run_bass_kernel_spmd
    ins, meta = preprocess(**inputs)
    # identity idx for output conv
    ident = _wrap16(np.arange(M, dtype=np.int64))
    for d in ins:
        d["idf"] = ident
    nc = build_program(meta)
    res = run_bass_kernel_spmd(nc, ins, core_ids=list(range(NC)))
    out = np.concatenate([res.results[s]["out"] for s in range(NC)], axis=0)
    return out.astype(np.float32)



# revision 11
# speedup vs baseline: 2.4190x; 2.4190x over previous
"""Trainium2 Bass kernel for nn_AffinityPredictor (sparse voxel GNN), v2.

Design (8 cores, N=65536 points, M=8192 own rows per core):
  - self-offset k=13 (56% of edges) computed as a dense per-tile matmul from
    the SBUF-resident table (no gather, no scatter).
  - non-self edges (~0.88/pt incl padding) gathered channel-major with an
    SBUF-source transposing dma_gather; per-k weight matmuls write Z^T to
    PSUM over exact (unpadded) free ranges; Z transposed back edge-major and
    staged to DRAM partition-major (big descriptors).
  - Z permuted to dst-tile order by a second (DRAM) dma_gather; one-hot Seg
    matmuls aggregate into per-tile PSUM together with the self term and
    BN statistics.
  - halo exchange instead of redundant compute: pre-BN park boundary strips
    are exchanged with neighbor cores via two pairwise AllGather collectives
    (staged uniformly, consumed with per-core host-computed gather indices),
    BN is then applied locally to own + halo rows; windows stay M+2*640 for
    every layer so one edge-list set serves all 9 layers.
  - tables and park live entirely in SBUF (two rotating table buffers; the
    residual source is overwritten stripe-by-stripe after its read).
"""
import numpy as np

N = 65536
NC = 8
M = N // NC
K = 27
KSELF = 13
CH = 256
CO = 128
EPS = 1e-5
NLAYER = 9
H = 640
W = M + 2 * H
NT = W // 128            # 74 table/park stripes
HT = H // 128            # 5 halo stripes per side
OT = M // 128            # 64 own tiles
GC = 896                 # gather idxs per call
PG = 256                 # psum group (edges) for conv Z^T
ZB = 16                  # z chunks per DRAM write batch
PB = 8                   # output write batch (tiles)

P1 = [[0, 1], [2, 3], [4, 5], [6, 7]]
P2 = [[0, 7], [1, 2], [3, 4], [5, 6]]
TORDER = (list(range(0, HT)) + list(range(OT - HT, OT))
          + list(range(HT, OT - HT)))


def _wrap16(idx):
    n = len(idx)
    assert n % 16 == 0, n
    a = np.asarray(idx)
    assert a.min() >= -32768 and a.max() < 32768, (a.min(), a.max())
    w = np.ascontiguousarray(a.astype(np.int16).reshape(-1, 16).T)
    return np.tile(w, (8, 1))


def _rup(x, m):
    return (x + m - 1) // m * m


def preprocess(feats, w_in, g_in, b_in, w_res, g_res, b_res, w_out,
               nbr_idx, nbr_mask):
    idx = np.asarray(nbr_idx)
    mask = np.asarray(nbr_mask).astype(bool)
    feats = np.asarray(feats, np.float32)
    f16 = np.float16

    delta = np.abs(idx.astype(np.int64) - np.arange(N)[:, None])[mask]
    assert delta.max() < H, delta.max()
    assert (idx[:, KSELF] == np.arange(N)).all() and mask[:, KSELF].all()

    ks_nonself = [k for k in range(K) if k != KSELF]

    srcs = [dict() for _ in range(NC)]
    dsts = [dict() for _ in range(NC)]
    for s in range(NC):
        lo = s * M - H
        own = np.arange(s * M, (s + 1) * M)
        mk = mask[own]
        ik = idx[own]
        for k in ks_nonself:
            sel = mk[:, k]
            srcs[s][k] = (ik[sel, k] - lo).astype(np.int64)
            dsts[s][k] = (own[sel] - s * M).astype(np.int64)

    ekmax = {k: max(len(srcs[s][k]) for s in range(NC)) for k in ks_nonself}
    n_exact = sum(ekmax.values())
    e1 = _rup(max(n_exact, 128), 128)
    nch = e1 // 128
    nz = 128 * nch

    kseg = []
    pos = 0
    for k in ks_nonself:
        kseg.append((k, pos, pos + ekmax[k]))
        pos += ekmax[k]

    g1 = np.zeros((NC, e1), np.int64)
    zdst = np.full((NC, e1), -1, np.int64)
    for s in range(NC):
        for (k, a, b) in kseg:
            sl, dl = srcs[s][k], dsts[s][k]
            g1[s, a:a + len(sl)] = sl
            zdst[s, a:a + len(dl)] = dl

    cnts = np.zeros((NC, OT), np.int64)
    for s in range(NC):
        zd = zdst[s]
        t = zd[zd >= 0] // 128
        np.add.at(cnts[s], t, 1)
    chunks = np.maximum((_rup(cnts.max(0), 128) // 128), 1)
    e2 = int(chunks.sum()) * 128
    g2 = np.zeros((NC, e2), np.int64)
    dof = np.full((NC, e2), -1.0, np.float16)
    for s in range(NC):
        zd = zdst[s]
        pertile = [np.nonzero(zd // 128 == t)[0] for t in range(OT)]
        off = 0
        for t in TORDER:
            rows = pertile[t]
            n_t = len(rows)
            cap = int(chunks[t]) * 128
            assert n_t <= cap
            g2[s, off:off + n_t] = (rows % 128) * nch + rows // 128
            dof[s, off:off + n_t] = (zd[rows] - t * 128).astype(f16)
            off += cap
        assert off == e2

    # halo exchange: phase A = pairwise AllGather (even boundaries),
    # phase B = all-8 AllGather (odd boundaries). Each core contributes ONE
    # mask-selected strip per phase: A: even->right, odd->left;
    # B: odd->right, even->left.
    # ccout rows [0,2H): A out (slot0 = lower rank); [2H,10H): B out slot s.
    hloL = np.zeros((NC, H), np.int64)
    hloR = np.zeros((NC, H), np.int64)
    for s in range(NC):
        if s > 0:
            nb = s - 1
            if nb % 2 == 0:           # even nb contributed right strip in A
                hloL[s] = 0 * H + np.arange(H)        # A slot 0 (nb < s)
            else:                      # odd nb contributed right strip in B
                hloL[s] = 2 * H + nb * H + np.arange(H)
        if s < NC - 1:
            nb = s + 1
            if nb % 2 == 1:           # odd nb contributed left strip in A
                hloR[s] = 1 * H + np.arange(H)        # A slot 1 (nb > s)
            else:                      # even nb contributed left strip in B
                hloR[s] = 2 * H + nb * H + np.arange(H)

    # conv matmul segments: cut k segments at GC-call and PG-psum grids
    cuts = set()
    for (k, a, b) in kseg:
        cuts.add(a); cuts.add(b)
    for c in range(0, e1, GC):
        cuts.add(c)
    for c in range(0, e1, PG):
        cuts.add(c)
    cuts.add(e1)
    cuts = sorted(cuts)
    segs = []
    for a, b in zip(cuts[:-1], cuts[1:]):
        k = None
        for (kk, ka, kb) in kseg:
            if ka <= a < kb:
                k = kk
                assert b <= kb
                break
        segs.append((k, a, b))

    meta = {"e1": e1, "e2": e2, "nch": nch, "nz": nz, "chunks": chunks,
            "kseg": kseg, "segs": segs, "ekmax": ekmax}

    w0 = np.zeros((K, 128, CH), f16)
    w0[:, :3, :] = np.asarray(w_in, np.float32).astype(f16)
    wr = np.asarray(w_res, np.float32).reshape(8, K, 2, 128, CH) \
        .transpose(0, 1, 3, 2, 4).astype(f16)
    wo = np.asarray(w_out, np.float32).reshape(2, 128, CO) \
        .transpose(1, 0, 2).astype(f16)
    bng = np.zeros((NLAYER, CH), np.float32)
    bnb = np.zeros((NLAYER, CH), np.float32)
    bng[0], bnb[0] = np.asarray(g_in), np.asarray(b_in)
    bng[1:] = np.asarray(g_res, np.float32).reshape(8, CH)
    bnb[1:] = np.asarray(b_res, np.float32).reshape(8, CH)

    ins = []
    for s in range(NC):
        lo = s * M - H
        t0 = np.zeros((W, 128), f16)
        a, b = max(0, lo), min(N, lo + W)
        t0[a - lo:b - lo, :3] = feats[a:b].astype(f16)
        ins.append({
            "w0": w0, "wr": wr, "wo": wo, "bng": bng, "bnb": bnb,
            "ones16": np.ones((128, 1), f16),
            "ones32": np.ones((1, 128), np.float32),
            "iota": np.tile(np.arange(128, dtype=f16), (128, 1)),
            "ident128": np.eye(128, dtype=f16),
            "tab0": np.ascontiguousarray(
                t0.reshape(NT, 128, 128).transpose(1, 0, 2)),
            "g1": _wrap16(g1[s]),
            "g2": _wrap16(g2[s]),
            "dof": np.ascontiguousarray(dof[s].reshape(-1, 128).T),
            "hloL": _wrap16(hloL[s]),
            "hloR": _wrap16(hloR[s]),
            # mask cols: [mAR, mAL, mBR, mBL]
            "msks": np.tile(np.array(
                [s % 2 == 0, s % 2 == 1, s % 2 == 1, s % 2 == 0],
                f16), (128, 1)),
        })
    return ins, meta


def build_program(meta):
    import os
    import concourse.tile as tile
    from concourse import bass, bacc, mybir
    from contextlib import ExitStack
    f16, f32, i16 = mybir.dt.float16, mybir.dt.float32, mybir.dt.int16
    AF = mybir.ActivationFunctionType
    OP = mybir.AluOpType

    klayers = int(os.environ.get("KLAYERS", str(NLAYER)))
    dbg = int(os.environ.get("DBG2", "0"))
    e1, e2, nch, nz = meta["e1"], meta["e2"], meta["nch"], meta["nz"]
    chunks, segs = meta["chunks"], meta["segs"]
    n_exact = sum(meta["ekmax"].values())

    nc = bacc.Bacc("TRN2", target_bir_lowering=False, debug=False,
                   num_devices=NC, num_swdge_queues=2)

    def din(name, shape, dt):
        return nc.dram_tensor(name, list(shape), dt, kind="ExternalInput").ap()

    w0_in = din("w0", (K, 128, CH), f16)
    wr_in = din("wr", (8, K, 128, 2, CH), f16)
    wo_in = din("wo", (128, 2, CO), f16)
    bng = din("bng", (NLAYER, CH), f32)
    bnb = din("bnb", (NLAYER, CH), f32)
    ones16 = din("ones16", (128, 1), f16)
    ones32 = din("ones32", (1, 128), f32)
    iota_in = din("iota", (128, 128), f16)
    ident_in = din("ident128", (128, 128), f16)
    tab0_in = din("tab0", (128, NT, 128), f16)
    g1_in = din("g1", (128, e1 // 16), i16)
    g2_in = din("g2", (128, e2 // 16), i16)
    dof_in = din("dof", (128, e2 // 128), f16)
    hloL_in = din("hloL", (128, H // 16), i16)
    hloR_in = din("hloR", (128, H // 16), i16)
    msks_in = din("msks", (128, 4), f16)

    out_t = nc.dram_tensor("out", [M, CO], f32, kind="ExternalOutput").ap()
    dbg_tabs = [nc.dram_tensor(f"dbgt{i}", [128, NT, CH], f16,
                               kind="ExternalOutput").ap()
                for i in range(NLAYER if dbg else 0)]
    dbg_parks = [nc.dram_tensor(f"dbgp{i}", [128, NT, CH], f16,
                                kind="ExternalOutput").ap()
                 for i in range(NLAYER if dbg else 0)]

    with tile.TileContext(nc) as tc, ExitStack() as ctx:
        sbc = ctx.enter_context(tc.tile_pool(name="sbc", bufs=1))
        gpool = ctx.enter_context(tc.tile_pool(name="gpool", bufs=3))
        zpool = ctx.enter_context(tc.tile_pool(name="zpool", bufs=3))
        wpool = ctx.enter_context(tc.tile_pool(name="wpool", bufs=3))
        spool = ctx.enter_context(tc.tile_pool(name="spool", bufs=3))
        xpool = ctx.enter_context(tc.tile_pool(name="xpool", bufs=3))
        ppool = ctx.enter_context(tc.tile_pool(name="ppool", bufs=2,
                                               space="PSUM"))
        ptp = ctx.enter_context(tc.tile_pool(name="ptp", bufs=2, space="PSUM"))
        pso = ctx.enter_context(tc.tile_pool(name="pso", bufs=2, space="PSUM"))
        pstat = ctx.enter_context(tc.tile_pool(name="pstat", bufs=1,
                                               space="PSUM"))
        dram = ctx.enter_context(tc.tile_pool(name="dram", bufs=1,
                                              space="DRAM"))

        # persistent constants / state
        onesK = sbc.tile([128, 1], f16)
        nc.sync.dma_start(out=onesK[:], in_=ones16[:])
        ones1 = sbc.tile([1, 128], f32)
        nc.sync.dma_start(out=ones1[:], in_=ones32[:])
        iota = sbc.tile([128, 128], f16)
        nc.sync.dma_start(out=iota[:], in_=iota_in[:])
        ident = sbc.tile([128, 128], f16)
        nc.sync.dma_start(out=ident[:], in_=ident_in[:])
        g1t = sbc.tile([128, e1 // 16], i16)
        nc.sync.dma_start(out=g1t[:], in_=g1_in[:])
        g2t = sbc.tile([128, e2 // 16], i16)
        nc.sync.dma_start(out=g2t[:], in_=g2_in[:])
        doft = sbc.tile([128, e2 // 128], f16)
        nc.sync.dma_start(out=doft[:], in_=dof_in[:])
        hloLt = sbc.tile([128, H // 16], i16)
        nc.sync.dma_start(out=hloLt[:], in_=hloL_in[:])
        hloRt = sbc.tile([128, H // 16], i16)
        nc.sync.dma_start(out=hloRt[:], in_=hloR_in[:])
        mskt = sbc.tile([128, 4], f16)
        nc.sync.dma_start(out=mskt[:], in_=msks_in[:])

        tabA = sbc.tile([128, NT, CH], f16)
        tabB = sbc.tile([128, NT, CH], f16)
        park = sbc.tile([128, NT, CH], f16)
        nc.sync.dma_start(out=tabA[:, :, 0:128], in_=tab0_in[:])

        zd = dram.tile([nz, CH], f16, name="zd")
        zd_v = zd.rearrange("(p c) f -> p c f", p=128)

        # stream position of each own tile (TORDER packing)
        toff = {}
        off = 0
        for t in TORDER:
            toff[t] = off
            off += int(chunks[t]) * 128

        for l in range(klayers):
            tabC = tabA if l % 2 == 0 else tabB
            tabN = tabB if l % 2 == 0 else tabA
            nblk = 1 if l == 0 else 2
            elem = 128 if l == 0 else CH
            last_l = (l == NLAYER - 1)

            # ---- conv phase: gathers ----
            gbufs = []
            for c0 in range(0, e1, GC):
                n = min(GC, e1 - c0)
                gb = gpool.tile([128, nblk, n], f16, tag="gb", name="gb")
                nc.gpsimd.dma_gather(
                    out_ap=gb[:], in_ap=tabC[:],
                    idxs_ap=g1t[:, c0 // 16:(c0 + n) // 16],
                    num_idxs=n, num_idxs_reg=n, elem_size=elem,
                    transpose=True, sbuf_tokens_per_rank=128,
                    sbuf_free_dim_per_rank=CH * 2)
                gbufs.append((c0, gb))

            # per-k weight tiles (load on first use)
            wk_tiles = {}

            def get_wk(k):
                if k not in wk_tiles:
                    wk = wpool.tile([128, nblk, CH], f16, tag="wk", name="wk")
                    if l == 0:
                        nc.sync.dma_start(out=wk[:, 0, :], in_=w0_in[k])
                    else:
                        nc.sync.dma_start(out=wk[:], in_=wr_in[l - 1, k])
                    wk_tiles[k] = wk
                return wk_tiles[k]

            # ---- conv phase: Z^T matmuls per psum group, transpose, stage ----
            zst = None
            zst_base = 0
            for g0 in range(0, e1, PG):
                gsegs = [(k, a, b) for (k, a, b) in segs
                         if g0 <= a < g0 + PG]
                pz = ppool.tile([128, 2, PG], f32, space="PSUM", tag="pz",
                                name="pz")
                for cb in range(2):
                    started = False
                    for (k, a, b) in gsegs:
                        if k is None:
                            continue
                        wk = get_wk(k)
                        call = a // GC
                        c0, gb = gbufs[call]
                        for ci in range(nblk):
                            nc.tensor.matmul(
                                out=pz[:, cb, a - g0:b - g0],
                                lhsT=wk[:, ci, cb * 128:(cb + 1) * 128],
                                rhs=gb[:, ci, a - c0:b - c0],
                                start=(ci == 0), stop=(ci == nblk - 1),
                                skip_group_check=True)
                        started = True
                zt16 = spool.tile([128, 2, PG], f16, tag="zt16", bufs=2, name="zt16")
                nc.any.tensor_copy(out=zt16[:, 0, :], in_=pz[:, 0, :])
                nc.any.tensor_copy(out=zt16[:, 1, :], in_=pz[:, 1, :])
                if g0 + PG > n_exact:
                    a = max(n_exact, g0)
                    nc.vector.memset(zt16[:, :, a - g0:], 0.0)
                if zst is None:
                    zst = zpool.tile([128, ZB, CH], f16, tag="zst", bufs=2, name="zst")
                    zst_base = g0 // 128
                for j in range(PG // 128):
                    slot = g0 // 128 + j - zst_base
                    for cb in range(2):
                        pt = ptp.tile([128, 128], f16, space="PSUM", tag="pt",
                                      name="pt")
                        nc.tensor.transpose(
                            out=pt[:], in_=zt16[:, cb, j * 128:(j + 1) * 128],
                            identity=ident[:])
                        nc.any.tensor_copy(
                            out=zst[:, slot, cb * 128:(cb + 1) * 128],
                            in_=pt[:])
                nslots = g0 // 128 + PG // 128 - zst_base
                if nslots == ZB or g0 + PG >= e1:
                    nc.sync.dma_start(
                        out=zd_v[:, zst_base:zst_base + nslots, :],
                        in_=zst[:, :nslots, :])
                    zst = None

            # ---- aggregation phase ----
            wself = wpool.tile([128, nblk, CH], f16, tag="wself", bufs=2,
                               name="wself")
            if l == 0:
                nc.sync.dma_start(out=wself[:, 0, :], in_=w0_in[KSELF])
            else:
                nc.sync.dma_start(out=wself[:], in_=wr_in[l - 1, KSELF])

            pst = pstat.tile([1, 2 * CH], f32, space="PSUM", tag="ps",
                             name="pst")
            zbufs = []

            def emit_g2():
                call = len(zbufs)
                c0 = call * GC
                n = min(GC, e2 - c0)
                zt = zpool.tile([128, GC // 128, CH], f16, tag="z2", name="zt")
                nc.gpsimd.dma_gather(
                    out_ap=zt[:, :n // 128, :], in_ap=zd[:],
                    idxs_ap=g2t[:, c0 // 16:(c0 + n) // 16],
                    num_idxs=n, num_idxs_reg=n, elem_size=CH, queue_num=1)
                zbufs.append(zt)

            ccinA = ccinB = ccout = None
            if not last_l:
                ccinA = dram.tile([H, CH], f16, name=f"ccinA{l}")
                ccinB = dram.tile([H, CH], f16, name=f"ccinB{l}")
                ccout = dram.tile([10 * H, CH], f16, name=f"ccout{l}")

            for i, t in enumerate(TORDER):
                po = pso.tile([128, CH], f32, space="PSUM", tag="po",
                              name="po")
                # self term
                for ci in range(nblk):
                    pt = ptp.tile([128, 128], f16, space="PSUM", tag="pt",
                                  name="pt")
                    nc.tensor.transpose(
                        out=pt[:],
                        in_=tabC[:, HT + t, ci * 128:(ci + 1) * 128],
                        identity=ident[:])
                    xt = xpool.tile([128, 128], f16, tag="xt", name="xt")
                    nc.any.tensor_copy(out=xt[:], in_=pt[:])
                    nc.tensor.matmul(out=po[:], lhsT=xt[:],
                                     rhs=wself[:, ci, :],
                                     start=(ci == 0), stop=False,
                                     skip_group_check=True)
                nchk = int(chunks[t])
                for c in range(nchk):
                    ci_g = toff[t] // 128 + c
                    call, slot = divmod(ci_g * 128, GC)
                    while len(zbufs) <= call:
                        emit_g2()
                    seg = spool.tile([128, 128], f16, tag="seg", name="seg")
                    nc.vector.tensor_tensor(
                        out=seg[:],
                        in0=doft[:, ci_g:ci_g + 1].to_broadcast([128, 128]),
                        in1=iota[:], op=OP.is_equal)
                    nc.tensor.matmul(out=po[:], lhsT=seg[:],
                                     rhs=zbufs[call][:, slot // 128, :],
                                     start=False, stop=(c == nchk - 1),
                                     skip_group_check=True)
                nc.any.tensor_copy(out=park[:, HT + t, :], in_=po[:])
                sq = spool.tile([128, CH], f16, tag="sq", name="sq")
                nc.scalar.activation(out=sq[:], in_=park[:, HT + t, :],
                                     func=AF.Square)
                nc.tensor.matmul(out=pst[:, :CH], lhsT=onesK[:],
                                 rhs=park[:, HT + t, :], start=(i == 0),
                                 stop=(i == OT - 1), skip_group_check=True)
                nc.tensor.matmul(out=pst[:, CH:], lhsT=onesK[:], rhs=sq[:],
                                 start=(i == 0), stop=(i == OT - 1),
                                 skip_group_check=True)
                if i == 2 * HT - 1 and not last_l:
                    # boundary tiles done: mask-select strips + post exchanges
                    for ph, ccin in ((0, ccinA), (1, ccinB)):
                        stg = spool.tile([128, HT, CH], f16, tag="hstg",
                                         bufs=2, name="stg")
                        mR = mskt[:, 2 * ph:2 * ph + 1].to_broadcast([128, CH])
                        mL = mskt[:, 2 * ph + 1:2 * ph + 2] \
                            .to_broadcast([128, CH])
                        for j in range(HT):
                            t1 = spool.tile([128, CH], f16, tag="ht1",
                                            name="t1")
                            nc.vector.tensor_tensor(
                                out=t1[:], in0=park[:, HT + OT - HT + j, :],
                                in1=mR, op=OP.mult)
                            t2 = spool.tile([128, CH], f16, tag="ht2",
                                            name="t2")
                            nc.vector.tensor_tensor(
                                out=t2[:], in0=park[:, HT + j, :],
                                in1=mL, op=OP.mult)
                            nc.vector.tensor_tensor(
                                out=stg[:, j, :], in0=t1[:], in1=t2[:],
                                op=OP.add)
                        ccin_v = ccin.rearrange("(b p) f -> p b f", p=128)
                        nc.sync.dma_start(out=ccin_v[:], in_=stg[:])
                    nc.gpsimd.collective_compute(
                        "AllGather", OP.bypass,
                        ins=[ccinA[:].opt()], outs=[ccout[0:2 * H, :].opt()],
                        replica_groups=P1)
                    nc.gpsimd.collective_compute(
                        "AllGather", OP.bypass,
                        ins=[ccinB[:].opt()],
                        outs=[ccout[2 * H:10 * H, :].opt()],
                        replica_groups=[list(range(NC))])

            # consume halo strips into park
            if not last_l:
                nc.gpsimd.dma_gather(
                    out_ap=park[:, 0:HT, :], in_ap=ccout[:],
                    idxs_ap=hloLt[:], num_idxs=H, num_idxs_reg=H,
                    elem_size=CH, queue_num=1)
                nc.gpsimd.dma_gather(
                    out_ap=park[:, HT + OT:NT, :], in_ap=ccout[:],
                    idxs_ap=hloRt[:], num_idxs=H, num_idxs_reg=H,
                    elem_size=CH, queue_num=1)

            # ---- stats allreduce + BN coefficients ----
            ccs_i = dram.tile([2, CH], f32, name=f"ccs_i{l}")
            ccs_o = dram.tile([2, CH], f32, addr_space="Shared",
                              name=f"ccs_o{l}")
            s1t = spool.tile([1, CH], f32, tag="s1t", bufs=1, name="s1t")
            nc.vector.tensor_copy(out=s1t[:], in_=pst[:, :CH])
            s2t = spool.tile([1, CH], f32, tag="s2t", bufs=1, name="s2t")
            nc.vector.tensor_copy(out=s2t[:], in_=pst[:, CH:])
            nc.sync.dma_start(out=ccs_i[0:1, :], in_=s1t[:])
            nc.sync.dma_start(out=ccs_i[1:2, :], in_=s2t[:])
            nc.gpsimd.collective_compute(
                "AllReduce", OP.add, ins=[ccs_i[:].opt()],
                outs=[ccs_o[:].opt()], replica_groups=[list(range(NC))])
            stg1 = spool.tile([1, CH], f32, tag="stg1", bufs=1, name="stg1")
            nc.sync.dma_start(out=stg1[:], in_=ccs_o[0:1, :])
            stg2 = spool.tile([1, CH], f32, tag="stg2", bufs=1, name="stg2")
            nc.sync.dma_start(out=stg2[:], in_=ccs_o[1:2, :])

            mu = spool.tile([1, CH], f32, tag="mu", bufs=1, name="mu")
            nc.vector.tensor_scalar_mul(out=mu[:], in0=stg1[:],
                                        scalar1=1.0 / N)
            var = spool.tile([1, CH], f32, tag="var", bufs=1, name="var")
            nc.vector.tensor_scalar_mul(out=var[:], in0=stg2[:],
                                        scalar1=1.0 / N)
            mu2 = spool.tile([1, CH], f32, tag="mu2", bufs=1, name="mu2")
            nc.vector.tensor_tensor(out=mu2[:], in0=mu[:], in1=mu[:],
                                    op=OP.mult)
            nc.vector.tensor_tensor(out=var[:], in0=var[:], in1=mu2[:],
                                    op=OP.subtract)
            nc.vector.tensor_scalar_add(out=var[:], in0=var[:], scalar1=EPS)
            sd = spool.tile([1, CH], f32, tag="sd", bufs=1, name="sd")
            nc.scalar.activation(out=sd[:], in_=var[:], func=AF.Sqrt)
            rsd = spool.tile([1, CH], f32, tag="rsd", bufs=1, name="rsd")
            nc.vector.reciprocal(out=rsd[:], in_=sd[:])
            q = spool.tile([1, CH], f32, tag="q", bufs=1, name="q")
            nc.vector.tensor_tensor(out=q[:], in0=rsd[:], in1=rsd[:],
                                    op=OP.mult)
            nc.vector.tensor_tensor(out=q[:], in0=q[:], in1=var[:],
                                    op=OP.mult)
            nc.vector.tensor_scalar(out=q[:], in0=q[:], scalar1=-0.5,
                                    scalar2=1.5, op0=OP.mult, op1=OP.add)
            nc.vector.tensor_tensor(out=rsd[:], in0=rsd[:], in1=q[:],
                                    op=OP.mult)
            gv = spool.tile([1, CH], f32, tag="gv", bufs=1, name="gv")
            nc.sync.dma_start(out=gv[:], in_=bng[l:l + 1, :])
            bv = spool.tile([1, CH], f32, tag="bv", bufs=1, name="bv")
            nc.sync.dma_start(out=bv[:], in_=bnb[l:l + 1, :])
            sc = spool.tile([1, CH], f32, tag="sc", bufs=1, name="sc")
            nc.vector.tensor_tensor(out=sc[:], in0=gv[:], in1=rsd[:],
                                    op=OP.mult)
            tc_ = spool.tile([1, CH], f32, tag="tc_", bufs=1, name="tc_")
            nc.vector.tensor_tensor(out=tc_[:], in0=mu[:], in1=sc[:],
                                    op=OP.mult)
            nc.vector.tensor_tensor(out=tc_[:], in0=bv[:], in1=tc_[:],
                                    op=OP.subtract)
            pbc = pso.tile([128, CH], f32, space="PSUM", tag="po", name="pbc")
            nc.tensor.matmul(out=pbc[:], lhsT=ones1[:], rhs=sc[:],
                             start=True, stop=True)
            scb = spool.tile([128, CH], f16, tag="scb", bufs=1, name="scb")
            nc.any.tensor_copy(out=scb[:], in_=pbc[:])
            pbc2 = pso.tile([128, CH], f32, space="PSUM", tag="po",
                            name="pbc2")
            nc.tensor.matmul(out=pbc2[:], lhsT=ones1[:], rhs=tc_[:],
                             start=True, stop=True)
            tcb = spool.tile([128, CH], f16, tag="tcb", bufs=1, name="tcb")
            nc.any.tensor_copy(out=tcb[:], in_=pbc2[:])

            # ---- BN apply: park -> tabN (overwrites residual source) ----
            is_res = (l >= 1) and (l % 2 == 0)
            stripes = range(NT) if not last_l else range(HT, HT + OT)
            for t in stripes:
                tmp = spool.tile([128, CH], f16, tag="tmp", name="tmp")
                nc.vector.tensor_tensor(out=tmp[:], in0=park[:, t, :],
                                        in1=scb[:], op=OP.mult)
                nc.vector.tensor_tensor(out=tmp[:], in0=tmp[:], in1=tcb[:],
                                        op=OP.add)
                if is_res:
                    nc.vector.tensor_tensor(out=tmp[:], in0=tmp[:],
                                            in1=tabN[:, t, :], op=OP.add)
                nc.scalar.activation(out=tabN[:, t, :], in_=tmp[:],
                                     func=AF.Relu)
            if dbg:
                nc.sync.dma_start(out=dbg_parks[l][:], in_=park[:])
                nc.sync.dma_start(out=dbg_tabs[l][:], in_=tabN[:])

        # ---- output conv ----
        if klayers == NLAYER:
            tabF = tabB if NLAYER % 2 == 1 else tabA
            wof = sbc.tile([128, 2, CO], f16)
            nc.sync.dma_start(out=wof[:], in_=wo_in[:])
            out_v = out_t.rearrange("(b p) f -> p b f", p=128)
            ost = None
            ost_base = 0
            for t in range(OT):
                po2 = pso.tile([128, CO], f32, space="PSUM", tag="po",
                               name="po2")
                for ci in range(2):
                    pt = ptp.tile([128, 128], f16, space="PSUM", tag="pt",
                                  name="pt")
                    nc.tensor.transpose(
                        out=pt[:],
                        in_=tabF[:, HT + t, ci * 128:(ci + 1) * 128],
                        identity=ident[:])
                    xt = xpool.tile([128, 128], f16, tag="xt", name="xt")
                    nc.any.tensor_copy(out=xt[:], in_=pt[:])
                    nc.tensor.matmul(out=po2[:], lhsT=xt[:],
                                     rhs=wof[:, ci, :], start=(ci == 0),
                                     stop=(ci == 1), skip_group_check=True)
                if ost is None:
                    ost = zpool.tile([128, PB, CO], f32, tag="ost", bufs=1,
                                     name="ost")
                    ost_base = t
                nc.any.tensor_copy(out=ost[:, t - ost_base, :], in_=po2[:])
                if t - ost_base == PB - 1 or t == OT - 1:
                    nc.sync.dma_start(
                        out=out_v[:, ost_base:t + 1, :],
                        in_=ost[:, :t - ost_base + 1, :])
                    ost = None

    nc.finalize()
    return nc


def kernel(**inputs):
    from concourse.bass_utils import run_bass_kernel_spmd
    ins, meta = preprocess(**inputs)
    nc = build_program(meta)
    res = run_bass_kernel_spmd(nc, ins, core_ids=list(range(NC)))
    out = np.concatenate([res.results[s]["out"] for s in range(NC)], axis=0)
    return out.astype(np.float32)
